# revision 15
# baseline (speedup 1.0000x reference)
"""Trainium2 Bass kernel for a dense transformer encoder layer.

Shapes (hardcoded): B=2, L=2048, D=1024, F=4096, H=16 heads, dk=64.
Sharding over 8 NeuronCores: core c handles batch b=c//4 and query-row
quarter r=c%4 (512 rows). K/V projections for the full batch are computed
per core (replicated within the 4-core batch group) so no collectives are
needed.

Key structure:
- Q/K/V projections in fp8 DoubleRow (contraction pairs packed 2/partition).
- RoPE uses an interleaved pair layout (projection output dims permuted so
  rotation partners sit on adjacent partitions); the half-rotation partition
  swap is a single DVE stream_shuffle (mask i^1) instead of SBUF DMAs.
  q/k land in bf16. Rope work is spread over three engines: psum->bf16 copy
  on the scalar engine, the two table muls + shuffle on the DVE, the final
  add on gpsimd.
- Scores bf16 matmuls; exp split between the scalar engine (native Exp ->
  fp8) and the DVE (Schraudolph bit-trick: one tensor_scalar f32->int8
  round-to-nearest, bitcast as fp8e4m3), weighted to late pairs where the
  DVE's rope work has drained.
- Softmax denominators via a ones-column in v_ext; reciprocal computed in a
  [128,8] layout (DMA gather/scatter) so the DVE reciprocal is ~60ns not 3us.
- w_o runs fp8 DoubleRow with weights pre-scaled x64 (ctx pre-scaled x4 via
  the onehot broadcast); the FFN runs bf16 (fp8 there blows the 2e-2 error
  gate, and LDWEIGHTS cannot be double-buffered in DR mode so compensated
  fp8 schemes cost the same as bf16 anyway).
- Phases 1+2 are software-pipelined with a deadline-scheduled filler queue.
"""
import os
import sys
import types

sys.path.insert(0, "/opt/trn_rl_repo")

import numpy as np
import ml_dtypes

import concourse.bass as bass
import concourse.tile as tile
import concourse.mybir as mybir
from contextlib import ExitStack

f32 = mybir.dt.float32
bf16 = mybir.dt.bfloat16
fp8 = mybir.dt.float8e4
i8 = mybir.dt.int8
AF = mybir.ActivationFunctionType
ALU = mybir.AluOpType
DR = mybir.MatmulPerfMode.DoubleRow

B, L, D, F, H, DK = 2, 2048, 1024, 4096, 16, 64
RQ = 512          # query rows per core
NCORES = 8
EPS = 1e-6

KT = D // 128     # 8 contraction tiles over D
KP = KT // 2      # 4 fp8 DoubleRow k-pairs over D
NL = L // 512     # 4 free chunks over L
LT = L // 128     # 16 l-tiles
KTP = LT // 2     # 8 key-tile pairs (ctx DoubleRow)
FT = F // 128     # 32 f-tiles
HP = H // 2       # 8 head pairs
VSTR = 65         # per-head stride in v_ext (64 v cols + ones)

WSCALE = 64.0     # fp8 weight pre-scale for w_o
CTXS = 4.0        # ctx pre-scale folded into onehot
EXP_C1 = 1.44269504   # Schraudolph: i8 = round(s*C1 + C2), bitcast fp8e4m3
EXP_C2 = 55.63
XMASK = [i ^ 1 for i in range(32)]   # stream_shuffle rope-partner swap

# units whose exp runs on the DVE (Schraudolph) instead of the scalar engine
# (u in 0..15 within each pair; late pairs lean on the DVE since the rope
# work that shares it is front-loaded)
DVE_EXP = {0: set(), 1: set(), 2: {5, 13}, 3: {5, 13},
           4: {3, 7, 11, 15}, 5: {3, 7, 11, 15},
           6: {1, 3, 5, 7, 9, 11, 13, 15},
           7: {1, 3, 5, 7, 9, 11, 13, 15}}

_PATCHED = False


def _install_patches():
    """Register the NTFF profile hook (if available) and wrap the BIR
    compile step to split multi-wait instructions (this walrus build
    accepts at most one sync-wait per instruction)."""
    global _PATCHED
    if _PATCHED:
        return
    _PATCHED = True

    if "antenv.axon_hooks" not in sys.modules:
        try:
            from trn_agent_boot.trn_boot import _ntff_profile_via_ctypes
            hook = _ntff_profile_via_ctypes("/opt/axon/libaxon_pjrt.so")
        except Exception:
            hook = None
        mod = types.ModuleType("antenv.axon_hooks")
        mod.get_axon_ntff_profile_hook = lambda: hook
        mod.set_axon_ntff_profile_hook = lambda h: None
        sys.modules["antenv.axon_hooks"] = mod

    import json

    def _split_multiwaits(bir_bytes):
        d = json.loads(bir_bytes)
        ctr = 0
        for fn in d.get("functions", []):
            for blk in fn.get("blocks", []):
                out = []
                for inst in blk.get("instructions", []):
                    si = inst.get("sync_info")
                    ow = (si or {}).get("on_wait") or []
                    if len(ow) > 1 and inst.get("engine", "Unassigned") != "Unassigned":
                        for w in ow[:-1]:
                            out.append({
                                "debug": inst.get("debug", 0),
                                "engine": inst["engine"],
                                "ins": [], "outs": [],
                                "name": f"I-antsw{ctr}",
                                "opcode": "NoOp",
                                "sync_info": {"on_update": [], "on_wait": [w]},
                            })
                            ctr += 1
                        si["on_wait"] = [ow[-1]]
                    out.append(inst)
                blk["instructions"] = out
        return json.dumps(d).encode()

    import concourse.bass_utils as bu
    import concourse.bass2jax as b2j

    orig = bu.compile_bir_kernel

    def patched(bir_json, tmpdir, neff_name="file.neff"):
        return orig(_split_multiwaits(bir_json), tmpdir, neff_name=neff_name)

    bu.compile_bir_kernel = patched
    b2j.compile_bir_kernel = patched


def _build_program(flags):
    """Build the SPMD Bass program (same NEFF for all 8 cores)."""
    nc = bass.Bass("TRN2", target_bir_lowering=False, debug=False,
                   num_devices=NCORES)

    def din(name, shape, dt):
        return nc.dram_tensor(name, shape, dt, kind="ExternalInput").ap()

    # fp8 inputs pre-arranged for DoubleRow: [kp, 128, 2, cols]
    xT8 = din("xT8", [KP, 128, 2, L], fp8)     # x[b].T, D-major k-pairs
    xq8 = din("xq8", [KP, 128, 2, RQ], fp8)    # this core's cols of x[b].T
    wq8 = din("wq8", [KP, 128, 2, D], fp8)     # cols interleave-permuted
    wk8 = din("wk8", [KP, 128, 2, D], fp8)     # cols interleave-permuted
    wv8 = din("wv8", [KP, 128, 2, D], fp8)
    xr = din("xr", [RQ, D], bf16)              # this core's rows (residual)
    cosr = din("cosr", [128, L], bf16)         # cos table (interleave layout)
    sinr = din("sinr", [128, L], bf16)         # sign-baked sin table
    qcos = din("qcos", [128, RQ], bf16)        # cos slice for this core's rows
    qsin = din("qsin", [128, RQ], bf16)
    wo8 = din("wo8", [KP, 128, 2, D], fp8)     # w_o * 64, DR-packed
    w1b = din("w1b", [D, F], bf16)
    w2b = din("w2b", [F, D], bf16)
    b1t = din("b1t", [128, F // 128], f32)     # b1 reshaped per-partition
    identb = din("identb", [128, 128], bf16)
    onehot = din("onehot", [H, H * 64], bf16)  # * CTXS
    bo = din("bo", [1, D], f32)
    b2r = din("b2r", [1, D], f32)
    g1 = din("g1", [1, D], f32)
    be1 = din("be1", [1, D], f32)
    g2 = din("g2", [1, D], f32)
    be2 = din("be2", [1, D], f32)
    y = nc.dram_tensor("y", [RQ, D], f32, kind="ExternalOutput").ap()

    def bcast_ap(ap2d, width):
        return bass.AP(tensor=ap2d.tensor, offset=ap2d.offset,
                       ap=[[0, 128], [1, width]])

    def dram_ap(t, offset, dims):
        return bass.AP(tensor=t.tensor, offset=t.offset + offset, ap=dims)

    with tile.TileContext(nc) as tc:
      with ExitStack() as top:
        # ---- pool stack (open order = reverse close order) ----
        consts = top.enter_context(tc.tile_pool(name="consts", bufs=1))
        poolW1 = top.enter_context(tc.tile_pool(name="pw1", bufs=1))  # w1
        poolWO = top.enter_context(tc.tile_pool(name="pwo", bufs=1))
        poolCT = top.enter_context(tc.tile_pool(name="pct", bufs=1))
        poolXR = top.enter_context(tc.tile_pool(name="pxr", bufs=1))

        # ---- long-lived constants (loads deferred to after the preloads) --
        identb_sb = consts.tile([128, 128], bf16, tag="identb", name="identb")
        b1_sb = consts.tile([128, F // 128], f32, tag="b1", name="b1")
        onehot_sb = consts.tile([H, H * 64], bf16, tag="onehot", name="onehot")
        eps_sb = consts.tile([128, 1], f32, tag="eps", name="eps")
        nc.vector.memset(eps_sb[:], EPS)

        rep_tiles = {}

        def rep_const(ap2d, use, tag):
            if not use:
                return None
            t = consts.tile([128, D], f32, tag=tag, name=tag)
            rep_tiles[tag] = (t, ap2d)
            return t

        bo_rep = rep_const(bo, flags["use_bo"], "bo")
        b2_rep = rep_const(b2r, flags["use_b2"], "b2")
        g1_rep = rep_const(g1, flags["use_g1"], "g1")
        be1_rep = rep_const(be1, flags["use_be1"], "be1")
        g2_rep = rep_const(g2, flags["use_g2"], "g2")
        be2_rep = rep_const(be2, flags["use_be2"], "be2")

        def load_consts():
            nc.sync.dma_start(identb_sb[:], identb[:])
            nc.sync.dma_start(b1_sb[:], b1t[:])
            nc.sync.dma_start(onehot_sb[:], onehot[:])
            for t, ap2d in rep_tiles.values():
                nc.sync.dma_start(out=t[:], in_=bcast_ap(ap2d, D))

        # ---- persistent activations (phase 1+2 lifetime) ----
        w1_sb = poolW1.tile([128, KT, F], bf16, tag="w1", name="w1")
        wo_sb = poolWO.tile([128, KP, 2, D], fp8, tag="wo", name="wo")
        ctx2 = poolCT.tile([128, KP, 2, RQ], fp8, tag="ctx2", name="ctx2")
        xr_sb = [poolXR.tile([128, D], bf16, tag=f"xr{t}", name=f"xr{t}")
                 for t in range(4)]

        # ============ Phases 1+2: pipelined projections + attention ========
        stackA = ExitStack()
        poolA = stackA.enter_context(tc.tile_pool(name="pa", bufs=1))
        epool = stackA.enter_context(tc.tile_pool(name="pe", bufs=3))
        crpool = stackA.enter_context(tc.tile_pool(name="pcr", bufs=1))
        ropew = stackA.enter_context(tc.tile_pool(name="prw", bufs=1))

        cos_sb = poolA.tile([128, L], bf16, tag="cos", name="cos")
        sin_sb = poolA.tile([128, L], bf16, tag="sin", name="sin")
        qcos_sb = poolA.tile([128, RQ], bf16, tag="qcos", name="qcos")
        qsin_sb = poolA.tile([128, RQ], bf16, tag="qsin", name="qsin")

        kTr = [poolA.tile([128, L], fp8, tag=f"kTr{m}", name=f"kTr{m}")
               for m in range(HP)]
        qTr = [poolA.tile([128, RQ], fp8, tag=f"qTr{m}", name=f"qTr{m}")
               for m in range(HP)]
        # v_ext[ktp][p, i, h*65+e] : L-row = ktp*256 + i*128 + p
        v_ext = [poolA.tile([128, 2, H * VSTR], fp8, tag=f"vx{t}",
                            name=f"vx{t}") for t in range(KTP)]

        stackP = ExitStack()
        poolX = stackP.enter_context(tc.tile_pool(name="px", bufs=1))
        wq_sb = poolX.tile([128, KP, 2, D], fp8, tag="wq", name="wq")
        xq_sb = poolX.tile([128, KP, 2, RQ], fp8, tag="xq", name="xq")
        xT_sb = poolX.tile([128, KP, 2, L], fp8, tag="xT", name="xT")
        wk_sb = poolX.tile([128, KP, 2, D], fp8, tag="wk", name="wk")
        wv_sb = poolX.tile([128, KP, 2, D], fp8, tag="wv", name="wv")

        # --- batched preloads, first-needed-first, split across 2 queues ---
        def load_w_dr(dst_slice, src, kp_lo, kp_n, col_lo, col_n, ncols, eng):
            # dst_slice: sbuf view [128, kp_n, 2, col_n]; src [KP,128,2,ncols]
            src_ap = dram_ap(
                src, kp_lo * 128 * 2 * ncols + col_lo,
                [[2 * ncols, 128], [128 * 2 * ncols, kp_n], [ncols, 2],
                 [1, col_n]])
            eng.dma_start(dst_slice, src_ap)

        # queue split: sync = q-side + late chunks; gpsimd = k-side firsts;
        # vector = xT chunk 0 (DVE idle until the first rope mul)
        for kp in range(KP):
            load_w_dr(wq_sb[:, kp, :, 0:256], wq8, kp, 1, 0, 256, D, nc.sync)
            load_w_dr(wk_sb[:, kp, :, 0:128], wk8, kp, 1, 0, 128, D,
                      nc.gpsimd)
            load_w_dr(xT_sb[:, kp, :, 0:512], xT8, kp, 1, 0, 512, L,
                      nc.scalar)
        load_w_dr(xq_sb[:], xq8, 0, KP, 0, RQ, RQ, nc.sync)
        nc.sync.dma_start(qcos_sb[:], qcos[:])
        nc.sync.dma_start(qsin_sb[:], qsin[:])
        nc.gpsimd.dma_start(cos_sb[:, 0:512], cosr[:, 0:512])
        nc.gpsimd.dma_start(sin_sb[:, 0:512], sinr[:, 0:512])
        for kp in range(KP):   # first v_unit needs wv cols 0:512
            load_w_dr(wv_sb[:, kp, :, 0:512], wv8, kp, 1, 0, 512, D,
                      nc.gpsimd)
        for kp in range(KP):
            load_w_dr(wq_sb[:, kp, :, 256:D], wq8, kp, 1, 256, D - 256, D,
                      nc.sync)
        for n in range(1, NL):
            nsl = slice(n * 512, n * 512 + 512)
            eng = nc.scalar if n == 1 else (nc.sync if n == 2 else nc.gpsimd)
            for kp in range(KP):
                load_w_dr(xT_sb[:, kp, :, nsl], xT8, kp, 1, n * 512, 512, L,
                          eng)
            eng.dma_start(cos_sb[:, nsl], cosr[:, nsl])
            eng.dma_start(sin_sb[:, nsl], sinr[:, nsl])
        for kp in range(KP):
            load_w_dr(wk_sb[:, kp, :, 128:D], wk8, kp, 1, 128, D - 128, D,
                      nc.sync)
            load_w_dr(wv_sb[:, kp, :, 512:1024], wv8, kp, 1, 512, 512, D,
                      nc.gpsimd)
        load_consts()

        w1_loaded = [False] * KT

        def load_w1(kt):
            if not w1_loaded[kt]:
                w1_loaded[kt] = True
                src = dram_ap(w1b, kt * 128 * F, [[F, 128], [1, F]])
                nc.sync.dma_start(w1_sb[:, kt, :], src)

        wo_loaded = [False]

        def load_wo():
            if not wo_loaded[0]:
                wo_loaded[0] = True
                load_w_dr(wo_sb[:], wo8, 0, KP, 0, D, D, nc.sync)

        xr_loaded = [False] * 4

        def load_xr(t):
            if not xr_loaded[t]:
                xr_loaded[t] = True
                nc.sync.dma_start(xr_sb[t][:], xr[t * 128:(t + 1) * 128, :])

        # psum pools for phases 1+2 (2 + 4 + 2 = 8 banks exactly)
        ppj = stackA.enter_context(tc.tile_pool(name="ppj", bufs=1,
                                                space="PSUM"))
        psc = stackA.enter_context(tc.tile_pool(name="psc", bufs=2,
                                                space="PSUM"))
        pcx = stackA.enter_context(tc.tile_pool(name="pcx", bufs=1,
                                                space="PSUM"))

        def rope_chunk(ps, cos_sl, sinsw_sl, dst, eng_add):
            """dst = st*cos + shuffle_xor1(st*sinsw).
            Engine split: psum copy on scalar, muls+shuffle on DVE, final
            add on gpsimd (steady state) or DVE (latency-critical early)."""
            n = dst.shape[-1]
            st = ropew.tile([128, 1024], bf16, tag="st", name="st")
            nc.scalar.copy(st[:, :n], ps)
            tct = ropew.tile([128, 1024], bf16, tag="rtc", name="rtc")
            nc.vector.tensor_mul(tct[:, :n], st[:, :n], cos_sl)
            tsn = ropew.tile([128, 1024], bf16, tag="rtm", name="rtm")
            nc.vector.tensor_mul(tsn[:, :n], st[:, :n], sinsw_sl)
            tsw = ropew.tile([128, 1024], bf16, tag="tsw", name="tsw")
            nc.vector.stream_shuffle(tsw[:, :n], tsn[:, :n], XMASK)
            eng_add.tensor_add(dst, tct[:, :n], tsw[:, :n])

        # --- projection work units (each: 4 DR matmuls + consumer) ---
        def q_unit(m, eng_add=nc.gpsimd):
            msl = slice(m * 128, m * 128 + 128)
            ps = ppj.tile([128, 1024], f32, tag="pj", name="pj")
            for kp in range(KP):
                nc.tensor.matmul(ps[:, 0:RQ], wq_sb[:, kp, :, msl],
                                 xq_sb[:, kp, :, :], start=(kp == 0),
                                 stop=(kp == KP - 1), perf_mode=DR)
            rope_chunk(ps[:, 0:RQ], qcos_sb[:], qsin_sb[:], qTr[m][:],
                       eng_add)

        def k_unit(hp, nn, eng_add=nc.gpsimd):
            nsl = slice(nn * 1024, nn * 1024 + 1024)
            msl = slice(hp * 128, hp * 128 + 128)
            ps = ppj.tile([128, 1024], f32, tag="pj", name="pj")
            for kp in range(KP):
                for hh in range(2):
                    hsl = slice(nn * 1024 + hh * 512,
                                nn * 1024 + hh * 512 + 512)
                    nc.tensor.matmul(ps[:, hh * 512:hh * 512 + 512],
                                     wk_sb[:, kp, :, msl],
                                     xT_sb[:, kp, :, hsl], start=(kp == 0),
                                     stop=(kp == KP - 1), perf_mode=DR)
            rope_chunk(ps[:], cos_sb[:, nsl], sin_sb[:, nsl],
                       kTr[hp][:, nsl], eng_add)

        def v_unit(lt, half):
            """v rows L-tile lt, cols half*512..+512 (heads 8*half..)."""
            ktp, i = lt // 2, lt % 2
            tsl = slice(lt * 128, lt * 128 + 128)
            psw = ppj.tile([128, 1024], f32, tag="pj", name="pj")
            ps = psw[:, 0:512]
            for kp in range(KP):
                nc.tensor.matmul(
                    ps, xT_sb[:, kp, :, tsl],
                    wv_sb[:, kp, :, half * 512:half * 512 + 512],
                    start=(kp == 0), stop=(kp == KP - 1), perf_mode=DR)
            vx_view = v_ext[ktp][:].rearrange("p i (h e) -> p i h e", h=H)
            ps_view = ps.rearrange("p (h e) -> p h e", h=8)
            nc.vector.tensor_copy(
                vx_view[:, i, half * 8:half * 8 + 8, 0:DK], ps_view[:])
            nc.gpsimd.memset(
                vx_view[:, i, half * 8:half * 8 + 8, DK:DK + 1], 1.0)

        # prologue: the minimum needed to start attention pair 0
        q_unit(0, nc.vector)
        q_unit(1, nc.vector)
        k_unit(0, 0, nc.vector)

        # deadline-scheduled filler queue: (deadline_unit, fn); the
        # attention loop runs 128 units (8 pairs x 16 single-head units)
        fq = []
        fq.append((0, lambda: v_unit(0, 0)))
        fq.append((0, lambda: v_unit(1, 0)))
        fq.append((5, lambda: k_unit(0, 1, nc.vector)))
        for m in range(2, HP):
            fq.append((16 * m - 10, lambda m=m: q_unit(m)))
        for hp in range(1, HP):
            for nn in range(2):
                fq.append((max(0, 16 * hp + 8 * nn - 8),
                           lambda hp=hp, nn=nn: k_unit(hp, nn)))
        for lt in range(2, LT):
            fq.append((max(0, 2 * (lt // 2) - 1), lambda lt=lt: v_unit(lt, 0)))
        for lt in range(LT):
            fq.append((56 + 2 * (lt // 2), lambda lt=lt: v_unit(lt, 1)))
        for kt in range(KT):
            fq.append((20 + 5 * kt, lambda kt=kt: load_w1(kt)))
        fq.append((66, load_wo))
        for t in range(4):
            fq.append((100 + 4 * t, lambda t=t: load_xr(t)))
        fq.sort(key=lambda t: t[0])

        def run_fillers(it, cap=2):
            # one unit if due within 4 iters; a second only if overdue
            n = 0
            while fq and n < cap:
                if fq[0][0] > (it + 4 if n == 0 else it):
                    break
                fq.pop(0)[1]()
                n += 1

        def rescale_a(hp, cpsA, cpsB):
            """Vector/DMA half of the rescale: ctx psum -> crA/crB and the
            denominator reciprocal in a [128,8] layout (DMA gather/scatter
            keeps the DVE reciprocal off the critical path and tiny)."""
            crA = crpool.tile([VSTR, RQ], f32, tag="crA", name="crA")
            nc.vector.tensor_copy(crA[:], cpsA[:])
            crB = crpool.tile([VSTR, RQ], f32, tag="crB", name="crB")
            nc.vector.tensor_copy(crB[:], cpsB[:])
            rin = crpool.tile([128, 8], f32, tag="rin", name="rin")
            nc.gpsimd.dma_start(rin[:, 0:4], crA[64:65, :])
            nc.gpsimd.dma_start(rin[:, 4:8], crB[64:65, :])
            recb = crpool.tile([128, 8], bf16, tag="recb", name="recb")
            with nc.allow_low_precision(reason="softmax denom recip in bf16"):
                nc.vector.reciprocal(recb[:], rin[:])
            rb = crpool.tile([2, RQ], bf16, tag="rb", name="rb")
            nc.gpsimd.dma_start(rb[0:1, :], recb[:, 0:4])
            nc.gpsimd.dma_start(rb[1:2, :], recb[:, 4:8])
            return (hp, crA, crB, rb)

        def rescale_b(hp, crA, crB, rb):
            """Tensor half (denominator broadcast + scale into ctx2),
            issued several units later when rb is long since ready."""
            for h, cr in ((2 * hp, crA), (2 * hp + 1, crB)):
                half = h % 2
                rpw = ppj.tile([128, 1024], f32, tag="pj", name="pj")
                rp = rpw[:, 0:512]
                nc.tensor.matmul(rp[0:64, :],
                                 onehot_sb[0:2, half * 64:half * 64 + 64],
                                 rb[:], start=True, stop=True)
                dst = ctx2[half * 64:half * 64 + 64, hp // 2, hp % 2, :]
                nc.vector.tensor_mul(dst, cr[0:64, :], rp[0:64, :])

        # --- attention steady-state loop ---
        # unit u of pair hp: head = u%2 (0=A in partitions 0:64), ktp = u//2.
        # ctx matmuls are issued one unit late (pend) so the exp wait never
        # head-of-line-blocks the next unit's score matmuls.
        def issue_ctx(pend):
            php, phead, pktp, pe, pcps = pend
            hh = 2 * php + phead
            nc.tensor.matmul(
                pcps[:], v_ext[pktp][:, :, hh * VSTR:hh * VSTR + VSTR],
                pe[:], start=(pktp == 0), stop=(pktp == KTP - 1),
                perf_mode=DR)

        pends = []       # queue of (hp, head, ktp, e_tile, cps); depth 2
        finished = None  # (hp, cpsA, cpsB) awaiting rescale
        resc_mid = None  # rescale_a output awaiting rescale_b
        cpsA = cpsB = None
        for hp in range(HP):
            for u in range(2 * KTP):
                head, ktp = u % 2, u // 2
                it = hp * 2 * KTP + u
                psl = slice(64 * head, 64 * head + 64)
                sc = psc.tile([128, 2, RQ], f32, tag="sc", name="sc")
                for i in range(2):
                    ksl = slice(ktp * 256 + i * 128,
                                ktp * 256 + i * 128 + 128)
                    nc.tensor.matmul(sc[:, i, :], kTr[hp][psl, ksl],
                                     qTr[hp][psl, :], start=True, stop=True)
                e = epool.tile([128, 2, RQ], fp8, tag="e", name="e")
                if u in DVE_EXP[hp]:
                    nc.vector.tensor_scalar(
                        e[:].bitcast(i8), sc[:], EXP_C1, EXP_C2,
                        op0=ALU.mult, op1=ALU.add)
                else:
                    nc.scalar.activation(e[:], sc[:], AF.Exp, scale=0.125)
                if len(pends) >= 2:
                    issue_ctx(pends.pop(0))
                if u == 1 and finished is not None:
                    resc_mid = rescale_a(*finished)
                    finished = None
                if u == 8 and resc_mid is not None:
                    rescale_b(*resc_mid)
                    resc_mid = None
                if u == 0:
                    cpsA = pcx.tile([VSTR, RQ], f32, tag="cpsA", name="cpsA")
                    cpsB = pcx.tile([VSTR, RQ], f32, tag="cpsB", name="cpsB")
                pends.append((hp, head, ktp, e,
                              cpsA if head == 0 else cpsB))
                run_fillers(it)
            finished = (hp, cpsA, cpsB)
        # flush remaining ctx and rescale the last pair
        while pends:
            issue_ctx(pends.pop(0))
        rescale_b(*rescale_a(*finished))

        while fq:
            fq.pop(0)[1]()

        stackP.close()
        stackA.close()

        # late-lifetime activations (phase 3 .. 4b) in the freed space
        late = ExitStack()
        poolLate = late.enter_context(tc.tile_pool(name="plate", bufs=1))
        hT = poolLate.tile([128, KT, RQ], bf16, tag="hT", name="hT")
        h_keep = [poolLate.tile([128, D], bf16, tag=f"hk{t}", name=f"hk{t}")
                  for t in range(4)]
        ff1 = poolLate.tile([128, FT, RQ], bf16, tag="ff1", name="ff1")

        # ================= Phase 3: w_o + residual + LN1 =================
        def layer_norm(dst, src, g_rep, be_rep, wpool):
            sview = src.rearrange("p (s d) -> p s d", s=2)
            stats = wpool.tile([128, 2, 6], f32, tag="lnstats", name="lnstats")
            for s in range(2):
                nc.vector.bn_stats(stats[:, s, :], sview[:, s, :])
            mv = wpool.tile([128, 2], f32, tag="lnmv", name="lnmv")
            nc.vector.bn_aggr(mv[:], stats[:])
            std = wpool.tile([128, 1], f32, tag="lnstd", name="lnstd")
            nc.scalar.activation(std[:], mv[:, 1:2], AF.Sqrt, bias=eps_sb[:])
            rstd = wpool.tile([128, 1], f32, tag="lnrstd", name="lnrstd")
            nc.vector.reciprocal(rstd[:], std[:])
            nc.vector.tensor_scalar(dst, src, mv[:, 0:1], rstd[:],
                                    op0=ALU.subtract, op1=ALU.mult)
            if g_rep is not None:
                nc.vector.tensor_mul(dst, dst, g_rep[:])
            if be_rep is not None:
                nc.vector.tensor_add(dst, dst, be_rep[:])

        WO_SC = 1.0 / (WSCALE * CTXS)
        with tc.tile_pool(name="ph3w", bufs=2) as ph3w, \
             tc.tile_pool(name="aops", bufs=2, space="PSUM") as aops, \
             tc.tile_pool(name="tpps", bufs=4, space="PSUM") as tpps:

            def wo_chain(qt):
                qsl = slice(qt * 128, qt * 128 + 128)
                ps = aops.tile([128, D], f32, tag="ao", name="ao")
                for half in range(2):
                    osl = slice(half * 512, half * 512 + 512)
                    for kp in range(KP):
                        nc.tensor.matmul(ps[:, osl], ctx2[:, kp, :, qsl],
                                         wo_sb[:, kp, :, osl],
                                         start=(kp == 0), stop=(kp == KP - 1),
                                         perf_mode=DR)
                res = ph3w.tile([128, D], f32, tag="res", name="res")
                nc.vector.scalar_tensor_tensor(res[:], ps[:], WO_SC,
                                               xr_sb[qt][:],
                                               op0=ALU.mult, op1=ALU.add)
                if bo_rep is not None:
                    nc.vector.tensor_add(res[:], res[:], bo_rep[:])
                layer_norm(h_keep[qt][:], res[:], g1_rep, be1_rep, ph3w)

            def transposes(qt):
                qsl = slice(qt * 128, qt * 128 + 128)
                for m in range(KT):
                    tp = tpps.tile([128, 128], bf16, tag="tp", name="tp")
                    nc.tensor.transpose(
                        tp[:], h_keep[qt][:, m * 128:m * 128 + 128],
                        identb_sb[:])
                    if m % 2 == 0:
                        nc.vector.tensor_copy(hT[:, m, qsl], tp[:])
                    else:
                        nc.scalar.copy(hT[:, m, qsl], tp[:])

            # staggered so the PE always has a wo chain or transposes ready
            # while the DVE works through the LN stream
            wo_chain(0)
            wo_chain(1)
            transposes(0)
            wo_chain(2)
            transposes(1)
            wo_chain(3)
            transposes(2)
            transposes(3)

        # ================= Phase 4a: FFN up + ReLU =================
        stack4 = ExitStack()
        ph4b_pool = stack4.enter_context(tc.tile_pool(name="ph4b", bufs=1))
        w2_sb = ph4b_pool.tile([128, FT, D], bf16, tag="w2", name="w2")

        def load_w2(j):   # 4 f-tiles per DMA
            src = dram_ap(w2b, 4 * j * 128 * D,
                          [[D, 128], [128 * D, 4], [1, D]])
            nc.gpsimd.dma_start(w2_sb[:, 4 * j:4 * j + 4, :], src)

        with tc.tile_pool(name="f1ps", bufs=4, space="PSUM") as f1ps:
            for ft in range(FT):
                fsl = slice(ft * 128, ft * 128 + 128)
                ps = f1ps.tile([128, RQ], f32, tag="f1", name="f1")
                for kt in range(KT):
                    nc.tensor.matmul(ps[:], w1_sb[:, kt, fsl], hT[:, kt, :],
                                     start=(kt == 0), stop=(kt == KT - 1))
                dst = ff1[:, ft, :]
                if flags["use_b1"]:
                    nc.scalar.activation(dst, ps[:], AF.Relu,
                                         bias=b1_sb[:, ft:ft + 1])
                elif ft % 2 == 0:
                    nc.scalar.activation(dst, ps[:], AF.Relu)
                else:
                    nc.vector.tensor_scalar(dst, ps[:], 0.0, None,
                                            op0=ALU.max)
                if ft % 4 == 0:
                    load_w2(ft // 4)

        # ================= Phase 4b: FFN down + LN2 =================
        with tc.tile_pool(name="ph4w", bufs=1) as ph4w, \
             tc.tile_pool(name="f2ps", bufs=3, space="PSUM") as f2ps:
            for qt in range(4):
                qsl = slice(qt * 128, qt * 128 + 128)
                ps = f2ps.tile([128, D], f32, tag="f2", name="f2")
                for half in range(2):
                    osl = slice(half * 512, half * 512 + 512)
                    for ft in range(FT):
                        nc.tensor.matmul(ps[:, osl], ff1[:, ft, qsl],
                                         w2_sb[:, ft, osl],
                                         start=(ft == 0), stop=(ft == FT - 1))
                res = ph4w.tile([128, D], f32, tag="res2", name="res2")
                nc.vector.tensor_add(res[:], ps[:], h_keep[qt][:])
                if b2_rep is not None:
                    nc.vector.tensor_add(res[:], res[:], b2_rep[:])
                layer_norm(res[:], res[:], g2_rep, be2_rep, ph4w)
                nc.sync.dma_start(y[qt * 128:(qt + 1) * 128, :], res[:])
        stack4.close()
        late.close()

    return nc


_CACHED = {}


def _get_program(flags):
    key = tuple(sorted(flags.items()))
    if key not in _CACHED:
        _CACHED[key] = _build_program(flags)
    return _CACHED[key]


def kernel(x, w_q, w_k, w_v, w_o, b_o, gamma1, beta1, gamma2, beta2,
           w1, b1, w2, b2, _trace=False):
    _install_patches()
    from concourse import bass_utils

    bf = ml_dtypes.bfloat16
    f8 = ml_dtypes.float8_e4m3
    x = np.asarray(x, np.float32)
    flags = {
        "use_bo": not np.all(np.asarray(b_o) == 0),
        "use_b1": not np.all(np.asarray(b1) == 0),
        "use_b2": not np.all(np.asarray(b2) == 0),
        "use_g1": not np.all(np.asarray(gamma1) == 1),
        "use_be1": not np.all(np.asarray(beta1) == 0),
        "use_g2": not np.all(np.asarray(gamma2) == 1),
        "use_be2": not np.all(np.asarray(beta2) == 0),
    }
    nc = _get_program(flags)

    # host-side shared prep
    inv_freq = (1.0 / (10000.0 ** (np.arange(0, DK, 2, dtype=np.float64) / DK)))
    freqs = np.arange(L, dtype=np.float64)[:, None] * inv_freq      # [L, 32]
    cos = np.cos(freqs).T.astype(np.float32)                        # [32, L]
    sin = np.sin(freqs).T.astype(np.float32)
    # interleaved pair layout: partition 2j <-> x1_j (+sin), 2j+1 <-> x2_j
    # (-sin); replicated for both heads of a pair (64-partition blocks)
    cos_i = np.empty((64, L), np.float32)
    sin_i = np.empty((64, L), np.float32)
    cos_i[0::2] = cos
    cos_i[1::2] = cos
    sin_i[0::2] = sin
    sin_i[1::2] = -sin
    cos_rep = np.tile(cos_i, (2, 1)).astype(bf)                     # [128, L]
    sin_sign = np.tile(sin_i, (2, 1)).astype(bf)
    # output-dim permutation for wq/wk: head h dim order
    # [x1_0, x2_0, x1_1, x2_1, ...]
    perm = np.empty(D, np.int64)
    for h in range(H):
        base = h * DK
        perm[base + 0:base + DK:2] = base + np.arange(32)
        perm[base + 1:base + DK:2] = base + 32 + np.arange(32)

    def pack_dr(w):  # [D, C] f32 -> [KP, 128, 2, C] fp8
        C = w.shape[1]
        return np.ascontiguousarray(
            w.reshape(KP, 2, 128, C).transpose(0, 2, 1, 3)).astype(f8)

    common = {
        "cosr": cos_rep, "sinr": sin_sign,
        "wq8": pack_dr(np.asarray(w_q, np.float32)[:, perm]),
        "wk8": pack_dr(np.asarray(w_k, np.float32)[:, perm]),
        "wv8": pack_dr(np.asarray(w_v, np.float32)),
        "wo8": pack_dr(np.asarray(w_o, np.float32) * WSCALE),
        "w1b": np.asarray(w1, np.float32).astype(bf),
        "w2b": np.asarray(w2, np.float32).astype(bf),
        "b1t": np.ascontiguousarray(
            np.asarray(b1, np.float32).reshape(F // 128, 128).T),
        "identb": np.eye(128, dtype=np.float32).astype(bf),
        "onehot": (np.kron(np.eye(H, dtype=np.float32),
                           np.ones((1, 64), np.float32)) * CTXS).astype(bf),
        "bo": np.asarray(b_o, np.float32).reshape(1, D),
        "b2r": np.asarray(b2, np.float32).reshape(1, D),
        "g1": np.asarray(gamma1, np.float32).reshape(1, D),
        "be1": np.asarray(beta1, np.float32).reshape(1, D),
        "g2": np.asarray(gamma2, np.float32).reshape(1, D),
        "be2": np.asarray(beta2, np.float32).reshape(1, D),
    }
    xT8_all = [pack_dr(np.ascontiguousarray(x[b].T)) for b in range(B)]

    in_maps = []
    for c in range(NCORES):
        b, r = c // 4, c % 4
        rows = slice(r * RQ, (r + 1) * RQ)
        m = dict(common)
        m["xT8"] = xT8_all[b]
        m["xq8"] = np.ascontiguousarray(xT8_all[b][:, :, :, rows])
        m["xr"] = np.ascontiguousarray(x[b, rows, :]).astype(bf)
        m["qcos"] = np.ascontiguousarray(cos_rep[:, rows])
        m["qsin"] = np.ascontiguousarray(sin_sign[:, rows])
        in_maps.append(m)

    res = bass_utils.run_bass_kernel_spmd(
        nc, in_maps, core_ids=list(range(NCORES)), trace=_trace)

    out = np.empty((B, L, D), np.float32)
    for c in range(NCORES):
        b, r = c // 4, c % 4
        out[b, r * RQ:(r + 1) * RQ, :] = res.results[c]["y"]
    if _trace:
        kernel.last_exec_time_ns = res.exec_time_ns
    return out


# revision 16
# speedup vs baseline: 1.0219x; 1.0219x over previous
"""Trainium2 Bass kernel for a dense transformer encoder layer.

Shapes (hardcoded): B=2, L=2048, D=1024, F=4096, H=16 heads, dk=64.
Sharding over 8 NeuronCores: core c handles batch b=c//4 and query-row
quarter r=c%4 (512 rows). K/V projections for the full batch are computed
per core (replicated within the 4-core batch group) so no collectives are
needed.

Key structure:
- Q/K/V projections in fp8 DoubleRow (contraction pairs packed 2/partition).
- RoPE uses an interleaved pair layout (projection output dims permuted so
  rotation partners sit on adjacent partitions); the half-rotation partition
  swap is a single DVE stream_shuffle (mask i^1) instead of SBUF DMAs.
  q/k land in bf16. Rope work is spread over three engines: psum->bf16 copy
  on the scalar engine, the two table muls + shuffle on the DVE, the final
  add on gpsimd.
- Scores bf16 matmuls; exp split between the scalar engine (native Exp ->
  fp8) and the DVE (Schraudolph bit-trick: one tensor_scalar f32->int8
  round-to-nearest, bitcast as fp8e4m3), weighted to late pairs where the
  DVE's rope work has drained.
- Softmax denominators via a ones-column in v_ext; reciprocal computed in a
  [128,8] layout (DMA gather/scatter) so the DVE reciprocal is ~60ns not 3us.
- w_o runs fp8 DoubleRow with weights pre-scaled x64 (ctx pre-scaled x4 via
  the onehot broadcast); the FFN runs bf16 (fp8 there blows the 2e-2 error
  gate, and LDWEIGHTS cannot be double-buffered in DR mode so compensated
  fp8 schemes cost the same as bf16 anyway).
- Phases 1+2 are software-pipelined with a deadline-scheduled filler queue.
"""
import os
import sys
import types

sys.path.insert(0, "/opt/trn_rl_repo")

import numpy as np
import ml_dtypes

import concourse.bass as bass
import concourse.tile as tile
import concourse.mybir as mybir
from contextlib import ExitStack

f32 = mybir.dt.float32
bf16 = mybir.dt.bfloat16
fp8 = mybir.dt.float8e4
i8 = mybir.dt.int8
AF = mybir.ActivationFunctionType
ALU = mybir.AluOpType
DR = mybir.MatmulPerfMode.DoubleRow

B, L, D, F, H, DK = 2, 2048, 1024, 4096, 16, 64
RQ = 512          # query rows per core
NCORES = 8
EPS = 1e-6

KT = D // 128     # 8 contraction tiles over D
KP = KT // 2      # 4 fp8 DoubleRow k-pairs over D
NL = L // 512     # 4 free chunks over L
LT = L // 128     # 16 l-tiles
KTP = LT // 2     # 8 key-tile pairs (ctx DoubleRow)
FT = F // 128     # 32 f-tiles
HP = H // 2       # 8 head pairs
VSTR = 65         # per-head stride in v_ext (64 v cols + ones)

WSCALE = 64.0     # fp8 weight pre-scale for w_o
CTXS = 4.0        # ctx pre-scale folded into onehot
EXP_C1 = 1.44269504   # Schraudolph: i8 = round(s*C1 + C2), bitcast fp8e4m3
EXP_C2 = 55.63
XMASK = [i ^ 1 for i in range(32)]   # stream_shuffle rope-partner swap

# units whose exp runs on the DVE (Schraudolph) instead of the scalar engine
# (u in 0..15 within each pair; late pairs lean on the DVE since the rope
# work that shares it is front-loaded)
DVE_EXP = {0: set(), 1: set(), 2: set(), 3: set(),
           4: {5, 13}, 5: {5, 13},
           6: {1, 3, 5, 7, 9, 11, 13, 15},
           7: {1, 3, 5, 7, 9, 11, 13, 15}}

_PATCHED = False


def _install_patches():
    """Register the NTFF profile hook (if available) and wrap the BIR
    compile step to split multi-wait instructions (this walrus build
    accepts at most one sync-wait per instruction)."""
    global _PATCHED
    if _PATCHED:
        return
    _PATCHED = True

    if "antenv.axon_hooks" not in sys.modules:
        try:
            from trn_agent_boot.trn_boot import _ntff_profile_via_ctypes
            hook = _ntff_profile_via_ctypes("/opt/axon/libaxon_pjrt.so")
        except Exception:
            hook = None
        mod = types.ModuleType("antenv.axon_hooks")
        mod.get_axon_ntff_profile_hook = lambda: hook
        mod.set_axon_ntff_profile_hook = lambda h: None
        sys.modules["antenv.axon_hooks"] = mod

    import json

    def _split_multiwaits(bir_bytes):
        d = json.loads(bir_bytes)
        ctr = 0
        for fn in d.get("functions", []):
            for blk in fn.get("blocks", []):
                out = []
                for inst in blk.get("instructions", []):
                    si = inst.get("sync_info")
                    ow = (si or {}).get("on_wait") or []
                    if len(ow) > 1 and inst.get("engine", "Unassigned") != "Unassigned":
                        for w in ow[:-1]:
                            out.append({
                                "debug": inst.get("debug", 0),
                                "engine": inst["engine"],
                                "ins": [], "outs": [],
                                "name": f"I-antsw{ctr}",
                                "opcode": "NoOp",
                                "sync_info": {"on_update": [], "on_wait": [w]},
                            })
                            ctr += 1
                        si["on_wait"] = [ow[-1]]
                    out.append(inst)
                blk["instructions"] = out
        return json.dumps(d).encode()

    import concourse.bass_utils as bu
    import concourse.bass2jax as b2j

    orig = bu.compile_bir_kernel

    def patched(bir_json, tmpdir, neff_name="file.neff"):
        return orig(_split_multiwaits(bir_json), tmpdir, neff_name=neff_name)

    bu.compile_bir_kernel = patched
    b2j.compile_bir_kernel = patched


def _build_program(flags):
    """Build the SPMD Bass program (same NEFF for all 8 cores)."""
    nc = bass.Bass("TRN2", target_bir_lowering=False, debug=False,
                   num_devices=NCORES)

    def din(name, shape, dt):
        return nc.dram_tensor(name, shape, dt, kind="ExternalInput").ap()

    # fp8 inputs pre-arranged for DoubleRow: [kp, 128, 2, cols]
    xT8 = din("xT8", [KP, 128, 2, L], fp8)     # x[b].T, D-major k-pairs
    xq8 = din("xq8", [KP, 128, 2, RQ], fp8)    # this core's cols of x[b].T
    wq8 = din("wq8", [KP, 128, 2, D], fp8)     # cols interleave-permuted
    wk8 = din("wk8", [KP, 128, 2, D], fp8)     # cols interleave-permuted
    wv8 = din("wv8", [KP, 128, 2, D], fp8)
    xr = din("xr", [RQ, D], bf16)              # this core's rows (residual)
    cosr = din("cosr", [128, L], bf16)         # cos table (interleave layout)
    sinr = din("sinr", [128, L], bf16)         # sign-baked sin table
    qcos = din("qcos", [128, RQ], bf16)        # cos slice for this core's rows
    qsin = din("qsin", [128, RQ], bf16)
    wo8 = din("wo8", [KP, 128, 2, D], fp8)     # w_o * 64, DR-packed
    w1b = din("w1b", [D, F], bf16)
    w2b = din("w2b", [F, D], bf16)
    b1t = din("b1t", [128, F // 128], f32)     # b1 reshaped per-partition
    identb = din("identb", [128, 128], bf16)
    onehot = din("onehot", [H, H * 64], bf16)  # * CTXS
    bo = din("bo", [1, D], f32)
    b2r = din("b2r", [1, D], f32)
    g1 = din("g1", [1, D], f32)
    be1 = din("be1", [1, D], f32)
    g2 = din("g2", [1, D], f32)
    be2 = din("be2", [1, D], f32)
    y = nc.dram_tensor("y", [RQ, D], f32, kind="ExternalOutput").ap()

    def bcast_ap(ap2d, width):
        return bass.AP(tensor=ap2d.tensor, offset=ap2d.offset,
                       ap=[[0, 128], [1, width]])

    def dram_ap(t, offset, dims):
        return bass.AP(tensor=t.tensor, offset=t.offset + offset, ap=dims)

    with tile.TileContext(nc) as tc:
      with ExitStack() as top:
        # ---- pool stack (open order = reverse close order) ----
        consts = top.enter_context(tc.tile_pool(name="consts", bufs=1))
        poolW1 = top.enter_context(tc.tile_pool(name="pw1", bufs=1))  # w1
        poolWO = top.enter_context(tc.tile_pool(name="pwo", bufs=1))
        poolCT = top.enter_context(tc.tile_pool(name="pct", bufs=1))
        poolXR = top.enter_context(tc.tile_pool(name="pxr", bufs=1))

        # ---- long-lived constants (loads deferred to after the preloads) --
        identb_sb = consts.tile([128, 128], bf16, tag="identb", name="identb")
        b1_sb = consts.tile([128, F // 128], f32, tag="b1", name="b1")
        onehot_sb = consts.tile([H, H * 64], bf16, tag="onehot", name="onehot")
        eps_sb = consts.tile([128, 1], f32, tag="eps", name="eps")
        nc.vector.memset(eps_sb[:], EPS)

        rep_tiles = {}

        def rep_const(ap2d, use, tag):
            if not use:
                return None
            t = consts.tile([128, D], f32, tag=tag, name=tag)
            rep_tiles[tag] = (t, ap2d)
            return t

        bo_rep = rep_const(bo, flags["use_bo"], "bo")
        b2_rep = rep_const(b2r, flags["use_b2"], "b2")
        g1_rep = rep_const(g1, flags["use_g1"], "g1")
        be1_rep = rep_const(be1, flags["use_be1"], "be1")
        g2_rep = rep_const(g2, flags["use_g2"], "g2")
        be2_rep = rep_const(be2, flags["use_be2"], "be2")

        def load_consts():
            nc.sync.dma_start(identb_sb[:], identb[:])
            nc.sync.dma_start(b1_sb[:], b1t[:])
            nc.sync.dma_start(onehot_sb[:], onehot[:])
            for t, ap2d in rep_tiles.values():
                nc.sync.dma_start(out=t[:], in_=bcast_ap(ap2d, D))

        # ---- persistent activations (phase 1+2 lifetime) ----
        w1_sb = poolW1.tile([128, KT, F], bf16, tag="w1", name="w1")
        wo_sb = poolWO.tile([128, KP, 2, D], fp8, tag="wo", name="wo")
        ctx2 = poolCT.tile([128, KP, 2, RQ], fp8, tag="ctx2", name="ctx2")
        xr_sb = [poolXR.tile([128, D], bf16, tag=f"xr{t}", name=f"xr{t}")
                 for t in range(4)]

        # ============ Phases 1+2: pipelined projections + attention ========
        stackA = ExitStack()
        poolA = stackA.enter_context(tc.tile_pool(name="pa", bufs=1))
        epool = stackA.enter_context(tc.tile_pool(name="pe", bufs=3))
        crpool = stackA.enter_context(tc.tile_pool(name="pcr", bufs=1))
        ropew = stackA.enter_context(tc.tile_pool(name="prw", bufs=1))

        cos_sb = poolA.tile([128, L], bf16, tag="cos", name="cos")
        sin_sb = poolA.tile([128, L], bf16, tag="sin", name="sin")
        qcos_sb = poolA.tile([128, RQ], bf16, tag="qcos", name="qcos")
        qsin_sb = poolA.tile([128, RQ], bf16, tag="qsin", name="qsin")

        kTr = [poolA.tile([128, L], fp8, tag=f"kTr{m}", name=f"kTr{m}")
               for m in range(HP)]
        qTr = [poolA.tile([128, RQ], fp8, tag=f"qTr{m}", name=f"qTr{m}")
               for m in range(HP)]
        # v_ext[ktp][p, i, h*65+e] : L-row = ktp*256 + i*128 + p
        v_ext = [poolA.tile([128, 2, H * VSTR], fp8, tag=f"vx{t}",
                            name=f"vx{t}") for t in range(KTP)]

        stackP = ExitStack()
        poolX = stackP.enter_context(tc.tile_pool(name="px", bufs=1))
        wq_sb = poolX.tile([128, KP, 2, D], fp8, tag="wq", name="wq")
        xq_sb = poolX.tile([128, KP, 2, RQ], fp8, tag="xq", name="xq")
        xT_sb = poolX.tile([128, KP, 2, L], fp8, tag="xT", name="xT")
        wk_sb = poolX.tile([128, KP, 2, D], fp8, tag="wk", name="wk")
        wv_sb = poolX.tile([128, KP, 2, D], fp8, tag="wv", name="wv")

        # --- batched preloads, first-needed-first, split across 2 queues ---
        def load_w_dr(dst_slice, src, kp_lo, kp_n, col_lo, col_n, ncols, eng):
            # dst_slice: sbuf view [128, kp_n, 2, col_n]; src [KP,128,2,ncols]
            src_ap = dram_ap(
                src, kp_lo * 128 * 2 * ncols + col_lo,
                [[2 * ncols, 128], [128 * 2 * ncols, kp_n], [ncols, 2],
                 [1, col_n]])
            eng.dma_start(dst_slice, src_ap)

        # queue split: sync = q-side + late chunks; gpsimd = k-side firsts;
        # vector = xT chunk 0 (DVE idle until the first rope mul)
        for kp in range(KP):
            load_w_dr(wq_sb[:, kp, :, 0:256], wq8, kp, 1, 0, 256, D, nc.sync)
            load_w_dr(wk_sb[:, kp, :, 0:128], wk8, kp, 1, 0, 128, D,
                      nc.gpsimd)
            load_w_dr(xT_sb[:, kp, :, 0:512], xT8, kp, 1, 0, 512, L,
                      nc.scalar)
        load_w_dr(xq_sb[:], xq8, 0, KP, 0, RQ, RQ, nc.sync)
        nc.sync.dma_start(qcos_sb[:], qcos[:])
        nc.sync.dma_start(qsin_sb[:], qsin[:])
        nc.gpsimd.dma_start(cos_sb[:, 0:512], cosr[:, 0:512])
        nc.gpsimd.dma_start(sin_sb[:, 0:512], sinr[:, 0:512])
        for kp in range(KP):   # first v_unit needs wv cols 0:512
            load_w_dr(wv_sb[:, kp, :, 0:512], wv8, kp, 1, 0, 512, D,
                      nc.gpsimd)
        for kp in range(KP):
            load_w_dr(wq_sb[:, kp, :, 256:D], wq8, kp, 1, 256, D - 256, D,
                      nc.sync)
        for n in range(1, NL):
            nsl = slice(n * 512, n * 512 + 512)
            eng = nc.scalar if n == 1 else (nc.sync if n == 2 else nc.gpsimd)
            for kp in range(KP):
                load_w_dr(xT_sb[:, kp, :, nsl], xT8, kp, 1, n * 512, 512, L,
                          eng)
            eng.dma_start(cos_sb[:, nsl], cosr[:, nsl])
            eng.dma_start(sin_sb[:, nsl], sinr[:, nsl])
        for kp in range(KP):
            load_w_dr(wk_sb[:, kp, :, 128:D], wk8, kp, 1, 128, D - 128, D,
                      nc.sync)
            load_w_dr(wv_sb[:, kp, :, 512:1024], wv8, kp, 1, 512, 512, D,
                      nc.gpsimd)
        load_consts()

        w1_loaded = [False] * KT

        def load_w1(kt):
            if not w1_loaded[kt]:
                w1_loaded[kt] = True
                src = dram_ap(w1b, kt * 128 * F, [[F, 128], [1, F]])
                nc.sync.dma_start(w1_sb[:, kt, :], src)

        wo_loaded = [False]

        def load_wo():
            if not wo_loaded[0]:
                wo_loaded[0] = True
                load_w_dr(wo_sb[:], wo8, 0, KP, 0, D, D, nc.sync)

        xr_loaded = [False] * 4

        def load_xr(t):
            if not xr_loaded[t]:
                xr_loaded[t] = True
                nc.sync.dma_start(xr_sb[t][:], xr[t * 128:(t + 1) * 128, :])

        # psum pools for phases 1+2 (2 + 4 + 2 = 8 banks exactly)
        ppj = stackA.enter_context(tc.tile_pool(name="ppj", bufs=1,
                                                space="PSUM"))
        psc = stackA.enter_context(tc.tile_pool(name="psc", bufs=2,
                                                space="PSUM"))
        pcx = stackA.enter_context(tc.tile_pool(name="pcx", bufs=1,
                                                space="PSUM"))

        def rope_chunk(ps, cos_sl, sinsw_sl, dst, eng_add):
            """dst = st*cos + shuffle_xor1(st*sinsw).
            Engine split: psum copy on scalar, muls+shuffle on DVE, final
            add on gpsimd (steady state) or DVE (latency-critical early)."""
            n = dst.shape[-1]
            st = ropew.tile([128, 1024], bf16, tag="st", name="st")
            nc.scalar.copy(st[:, :n], ps)
            tct = ropew.tile([128, 1024], bf16, tag="rtc", name="rtc")
            nc.vector.tensor_mul(tct[:, :n], st[:, :n], cos_sl)
            tsn = ropew.tile([128, 1024], bf16, tag="rtm", name="rtm")
            nc.vector.tensor_mul(tsn[:, :n], st[:, :n], sinsw_sl)
            tsw = ropew.tile([128, 1024], bf16, tag="tsw", name="tsw")
            nc.vector.stream_shuffle(tsw[:, :n], tsn[:, :n], XMASK)
            eng_add.tensor_add(dst, tct[:, :n], tsw[:, :n])

        # --- projection work units (each: 4 DR matmuls + consumer) ---
        def q_unit(m, eng_add=nc.gpsimd):
            msl = slice(m * 128, m * 128 + 128)
            ps = ppj.tile([128, 1024], f32, tag="pj", name="pj")
            for kp in range(KP):
                nc.tensor.matmul(ps[:, 0:RQ], wq_sb[:, kp, :, msl],
                                 xq_sb[:, kp, :, :], start=(kp == 0),
                                 stop=(kp == KP - 1), perf_mode=DR)
            rope_chunk(ps[:, 0:RQ], qcos_sb[:], qsin_sb[:], qTr[m][:],
                       eng_add)

        def k_unit(hp, nn, eng_add=nc.gpsimd):
            nsl = slice(nn * 1024, nn * 1024 + 1024)
            msl = slice(hp * 128, hp * 128 + 128)
            ps = ppj.tile([128, 1024], f32, tag="pj", name="pj")
            for kp in range(KP):
                for hh in range(2):
                    hsl = slice(nn * 1024 + hh * 512,
                                nn * 1024 + hh * 512 + 512)
                    nc.tensor.matmul(ps[:, hh * 512:hh * 512 + 512],
                                     wk_sb[:, kp, :, msl],
                                     xT_sb[:, kp, :, hsl], start=(kp == 0),
                                     stop=(kp == KP - 1), perf_mode=DR)
            rope_chunk(ps[:], cos_sb[:, nsl], sin_sb[:, nsl],
                       kTr[hp][:, nsl], eng_add)

        def v_unit(lt, half):
            """v rows L-tile lt, cols half*512..+512 (heads 8*half..)."""
            ktp, i = lt // 2, lt % 2
            tsl = slice(lt * 128, lt * 128 + 128)
            psw = ppj.tile([128, 1024], f32, tag="pj", name="pj")
            ps = psw[:, 0:512]
            for kp in range(KP):
                nc.tensor.matmul(
                    ps, xT_sb[:, kp, :, tsl],
                    wv_sb[:, kp, :, half * 512:half * 512 + 512],
                    start=(kp == 0), stop=(kp == KP - 1), perf_mode=DR)
            vx_view = v_ext[ktp][:].rearrange("p i (h e) -> p i h e", h=H)
            ps_view = ps.rearrange("p (h e) -> p h e", h=8)
            nc.vector.tensor_copy(
                vx_view[:, i, half * 8:half * 8 + 8, 0:DK], ps_view[:])
            nc.gpsimd.memset(
                vx_view[:, i, half * 8:half * 8 + 8, DK:DK + 1], 1.0)

        # prologue: the minimum needed to start attention pair 0
        q_unit(0, nc.vector)
        q_unit(1, nc.vector)
        k_unit(0, 0, nc.vector)

        # deadline-scheduled filler queue: (deadline_unit, fn); the
        # attention loop runs 128 units (8 pairs x 16 single-head units)
        fq = []
        fq.append((0, lambda: v_unit(0, 0)))
        fq.append((0, lambda: v_unit(1, 0)))
        fq.append((5, lambda: k_unit(0, 1, nc.vector)))
        for m in range(2, HP):
            fq.append((16 * m - 10, lambda m=m: q_unit(m)))
        for hp in range(1, HP):
            for nn in range(2):
                fq.append((max(0, 16 * hp + 8 * nn - 8),
                           lambda hp=hp, nn=nn: k_unit(hp, nn)))
        for lt in range(2, LT):
            fq.append((max(0, 2 * (lt // 2) - 1), lambda lt=lt: v_unit(lt, 0)))
        for lt in range(LT):
            fq.append((56 + 2 * (lt // 2), lambda lt=lt: v_unit(lt, 1)))
        for kt in range(KT):
            fq.append((20 + 5 * kt, lambda kt=kt: load_w1(kt)))
        fq.append((66, load_wo))
        for t in range(4):
            fq.append((100 + 4 * t, lambda t=t: load_xr(t)))
        fq.sort(key=lambda t: t[0])

        def run_fillers(it, cap=2):
            # one unit if due within 4 iters; a second only if overdue
            n = 0
            while fq and n < cap:
                if fq[0][0] > (it + 4 if n == 0 else it):
                    break
                fq.pop(0)[1]()
                n += 1

        def rescale_a(hp, cpsA, cpsB):
            """Vector/DMA half of the rescale: ctx psum -> crA/crB and the
            denominator reciprocal in a [128,8] layout (DMA gather/scatter
            keeps the DVE reciprocal off the critical path and tiny)."""
            crA = crpool.tile([VSTR, RQ], f32, tag="crA", name="crA")
            nc.vector.tensor_copy(crA[:], cpsA[:])
            crB = crpool.tile([VSTR, RQ], f32, tag="crB", name="crB")
            nc.vector.tensor_copy(crB[:], cpsB[:])
            rin = crpool.tile([128, 8], f32, tag="rin", name="rin")
            nc.gpsimd.dma_start(rin[:, 0:4], crA[64:65, :])
            nc.gpsimd.dma_start(rin[:, 4:8], crB[64:65, :])
            recb = crpool.tile([128, 8], bf16, tag="recb", name="recb")
            with nc.allow_low_precision(reason="softmax denom recip in bf16"):
                nc.vector.reciprocal(recb[:], rin[:])
            rb = crpool.tile([2, RQ], bf16, tag="rb", name="rb")
            nc.gpsimd.dma_start(rb[0:1, :], recb[:, 0:4])
            nc.gpsimd.dma_start(rb[1:2, :], recb[:, 4:8])
            return (hp, crA, crB, rb)

        def rescale_b(hp, crA, crB, rb):
            """Tensor half (denominator broadcast + scale into ctx2),
            issued several units later when rb is long since ready."""
            for h, cr in ((2 * hp, crA), (2 * hp + 1, crB)):
                half = h % 2
                rpw = ppj.tile([128, 1024], f32, tag="pj", name="pj")
                rp = rpw[:, 0:512]
                nc.tensor.matmul(rp[0:64, :],
                                 onehot_sb[0:2, half * 64:half * 64 + 64],
                                 rb[:], start=True, stop=True)
                dst = ctx2[half * 64:half * 64 + 64, hp // 2, hp % 2, :]
                nc.vector.tensor_mul(dst, cr[0:64, :], rp[0:64, :])

        # --- attention steady-state loop ---
        # unit u of pair hp: head = u%2 (0=A in partitions 0:64), ktp = u//2.
        # ctx matmuls are issued one unit late (pend) so the exp wait never
        # head-of-line-blocks the next unit's score matmuls.
        def issue_ctx(pend):
            php, phead, pktp, pe, pcps = pend
            hh = 2 * php + phead
            nc.tensor.matmul(
                pcps[:], v_ext[pktp][:, :, hh * VSTR:hh * VSTR + VSTR],
                pe[:], start=(pktp == 0), stop=(pktp == KTP - 1),
                perf_mode=DR)

        pends = []       # queue of (hp, head, ktp, e_tile, cps); depth 2
        finished = None  # (hp, cpsA, cpsB) awaiting rescale
        resc_mid = None  # rescale_a output awaiting rescale_b
        cpsA = cpsB = None
        for hp in range(HP):
            for u in range(2 * KTP):
                head, ktp = u % 2, u // 2
                it = hp * 2 * KTP + u
                psl = slice(64 * head, 64 * head + 64)
                sc = psc.tile([128, 2, RQ], f32, tag="sc", name="sc")
                for i in range(2):
                    ksl = slice(ktp * 256 + i * 128,
                                ktp * 256 + i * 128 + 128)
                    nc.tensor.matmul(sc[:, i, :], kTr[hp][psl, ksl],
                                     qTr[hp][psl, :], start=True, stop=True)
                e = epool.tile([128, 2, RQ], fp8, tag="e", name="e")
                if u in DVE_EXP[hp]:
                    nc.vector.tensor_scalar(
                        e[:].bitcast(i8), sc[:], EXP_C1, EXP_C2,
                        op0=ALU.mult, op1=ALU.add)
                else:
                    nc.scalar.activation(e[:], sc[:], AF.Exp, scale=0.125)
                if len(pends) >= 2:
                    issue_ctx(pends.pop(0))
                if u == 1 and finished is not None:
                    resc_mid = rescale_a(*finished)
                    finished = None
                if u == 8 and resc_mid is not None:
                    rescale_b(*resc_mid)
                    resc_mid = None
                if u == 0:
                    cpsA = pcx.tile([VSTR, RQ], f32, tag="cpsA", name="cpsA")
                    cpsB = pcx.tile([VSTR, RQ], f32, tag="cpsB", name="cpsB")
                pends.append((hp, head, ktp, e,
                              cpsA if head == 0 else cpsB))
                run_fillers(it)
            finished = (hp, cpsA, cpsB)
        # flush remaining ctx and rescale the last pair
        while pends:
            issue_ctx(pends.pop(0))
        rescale_b(*rescale_a(*finished))

        while fq:
            fq.pop(0)[1]()

        stackP.close()
        stackA.close()

        # late-lifetime activations (phase 3 .. 4b) in the freed space
        late = ExitStack()
        poolLate = late.enter_context(tc.tile_pool(name="plate", bufs=1))
        hT = poolLate.tile([128, KT, RQ], bf16, tag="hT", name="hT")
        h_keep = [poolLate.tile([128, D], bf16, tag=f"hk{t}", name=f"hk{t}")
                  for t in range(4)]
        ff1 = poolLate.tile([128, FT, RQ], bf16, tag="ff1", name="ff1")

        # ================= Phase 3: w_o + residual + LN1 =================
        def layer_norm(dst, src, g_rep, be_rep, wpool):
            sview = src.rearrange("p (s d) -> p s d", s=2)
            stats = wpool.tile([128, 2, 6], f32, tag="lnstats", name="lnstats")
            for s in range(2):
                nc.vector.bn_stats(stats[:, s, :], sview[:, s, :])
            mv = wpool.tile([128, 2], f32, tag="lnmv", name="lnmv")
            nc.vector.bn_aggr(mv[:], stats[:])
            std = wpool.tile([128, 1], f32, tag="lnstd", name="lnstd")
            nc.scalar.activation(std[:], mv[:, 1:2], AF.Sqrt, bias=eps_sb[:])
            rstd = wpool.tile([128, 1], f32, tag="lnrstd", name="lnrstd")
            nc.vector.reciprocal(rstd[:], std[:])
            nc.vector.tensor_scalar(dst, src, mv[:, 0:1], rstd[:],
                                    op0=ALU.subtract, op1=ALU.mult)
            if g_rep is not None:
                nc.vector.tensor_mul(dst, dst, g_rep[:])
            if be_rep is not None:
                nc.vector.tensor_add(dst, dst, be_rep[:])

        WO_SC = 1.0 / (WSCALE * CTXS)
        with tc.tile_pool(name="ph3w", bufs=2) as ph3w, \
             tc.tile_pool(name="aops", bufs=2, space="PSUM") as aops, \
             tc.tile_pool(name="tpps", bufs=4, space="PSUM") as tpps:

            def wo_chain(qt):
                qsl = slice(qt * 128, qt * 128 + 128)
                ps = aops.tile([128, D], f32, tag="ao", name="ao")
                for half in range(2):
                    osl = slice(half * 512, half * 512 + 512)
                    for kp in range(KP):
                        nc.tensor.matmul(ps[:, osl], ctx2[:, kp, :, qsl],
                                         wo_sb[:, kp, :, osl],
                                         start=(kp == 0), stop=(kp == KP - 1),
                                         perf_mode=DR)
                res = ph3w.tile([128, D], f32, tag="res", name="res")
                nc.vector.scalar_tensor_tensor(res[:], ps[:], WO_SC,
                                               xr_sb[qt][:],
                                               op0=ALU.mult, op1=ALU.add)
                if bo_rep is not None:
                    nc.vector.tensor_add(res[:], res[:], bo_rep[:])
                layer_norm(h_keep[qt][:], res[:], g1_rep, be1_rep, ph3w)

            def transposes(qt):
                qsl = slice(qt * 128, qt * 128 + 128)
                for m in range(KT):
                    tp = tpps.tile([128, 128], bf16, tag="tp", name="tp")
                    nc.tensor.transpose(
                        tp[:], h_keep[qt][:, m * 128:m * 128 + 128],
                        identb_sb[:])
                    if m % 2 == 0:
                        nc.vector.tensor_copy(hT[:, m, qsl], tp[:])
                    else:
                        nc.scalar.copy(hT[:, m, qsl], tp[:])

            # staggered so the PE always has a wo chain or transposes ready
            # while the DVE works through the LN stream
            wo_chain(0)
            wo_chain(1)
            transposes(0)
            wo_chain(2)
            transposes(1)
            wo_chain(3)
            transposes(2)
            transposes(3)

        # ================= Phase 4a: FFN up + ReLU =================
        stack4 = ExitStack()
        ph4b_pool = stack4.enter_context(tc.tile_pool(name="ph4b", bufs=1))
        w2_sb = ph4b_pool.tile([128, FT, D], bf16, tag="w2", name="w2")

        def load_w2(j):   # 4 f-tiles per DMA
            src = dram_ap(w2b, 4 * j * 128 * D,
                          [[D, 128], [128 * D, 4], [1, D]])
            nc.gpsimd.dma_start(w2_sb[:, 4 * j:4 * j + 4, :], src)

        with tc.tile_pool(name="f1ps", bufs=4, space="PSUM") as f1ps:
            for ft in range(FT):
                fsl = slice(ft * 128, ft * 128 + 128)
                ps = f1ps.tile([128, RQ], f32, tag="f1", name="f1")
                for kt in range(KT):
                    nc.tensor.matmul(ps[:], w1_sb[:, kt, fsl], hT[:, kt, :],
                                     start=(kt == 0), stop=(kt == KT - 1))
                dst = ff1[:, ft, :]
                if flags["use_b1"]:
                    nc.scalar.activation(dst, ps[:], AF.Relu,
                                         bias=b1_sb[:, ft:ft + 1])
                elif ft % 2 == 0:
                    nc.scalar.activation(dst, ps[:], AF.Relu)
                else:
                    nc.vector.tensor_scalar(dst, ps[:], 0.0, None,
                                            op0=ALU.max)
                if ft % 4 == 0:
                    load_w2(ft // 4)

        # ================= Phase 4b: FFN down + LN2 =================
        with tc.tile_pool(name="ph4w", bufs=1) as ph4w, \
             tc.tile_pool(name="f2ps", bufs=3, space="PSUM") as f2ps:
            for qt in range(4):
                qsl = slice(qt * 128, qt * 128 + 128)
                ps = f2ps.tile([128, D], f32, tag="f2", name="f2")
                for half in range(2):
                    osl = slice(half * 512, half * 512 + 512)
                    for ft in range(FT):
                        nc.tensor.matmul(ps[:, osl], ff1[:, ft, qsl],
                                         w2_sb[:, ft, osl],
                                         start=(ft == 0), stop=(ft == FT - 1))
                res = ph4w.tile([128, D], f32, tag="res2", name="res2")
                nc.vector.tensor_add(res[:], ps[:], h_keep[qt][:])
                if b2_rep is not None:
                    nc.vector.tensor_add(res[:], res[:], b2_rep[:])
                layer_norm(res[:], res[:], g2_rep, be2_rep, ph4w)
                nc.sync.dma_start(y[qt * 128:(qt + 1) * 128, :], res[:])
        stack4.close()
        late.close()

    return nc


_CACHED = {}


def _get_program(flags):
    key = tuple(sorted(flags.items()))
    if key not in _CACHED:
        _CACHED[key] = _build_program(flags)
    return _CACHED[key]


def kernel(x, w_q, w_k, w_v, w_o, b_o, gamma1, beta1, gamma2, beta2,
           w1, b1, w2, b2, _trace=False):
    _install_patches()
    from concourse import bass_utils

    bf = ml_dtypes.bfloat16
    f8 = ml_dtypes.float8_e4m3
    x = np.asarray(x, np.float32)
    flags = {
        "use_bo": not np.all(np.asarray(b_o) == 0),
        "use_b1": not np.all(np.asarray(b1) == 0),
        "use_b2": not np.all(np.asarray(b2) == 0),
        "use_g1": not np.all(np.asarray(gamma1) == 1),
        "use_be1": not np.all(np.asarray(beta1) == 0),
        "use_g2": not np.all(np.asarray(gamma2) == 1),
        "use_be2": not np.all(np.asarray(beta2) == 0),
    }
    nc = _get_program(flags)

    # host-side shared prep
    inv_freq = (1.0 / (10000.0 ** (np.arange(0, DK, 2, dtype=np.float64) / DK)))
    freqs = np.arange(L, dtype=np.float64)[:, None] * inv_freq      # [L, 32]
    cos = np.cos(freqs).T.astype(np.float32)                        # [32, L]
    sin = np.sin(freqs).T.astype(np.float32)
    # interleaved pair layout: partition 2j <-> x1_j (+sin), 2j+1 <-> x2_j
    # (-sin); replicated for both heads of a pair (64-partition blocks)
    cos_i = np.empty((64, L), np.float32)
    sin_i = np.empty((64, L), np.float32)
    cos_i[0::2] = cos
    cos_i[1::2] = cos
    sin_i[0::2] = sin
    sin_i[1::2] = -sin
    cos_rep = np.tile(cos_i, (2, 1)).astype(bf)                     # [128, L]
    sin_sign = np.tile(sin_i, (2, 1)).astype(bf)
    # output-dim permutation for wq/wk: head h dim order
    # [x1_0, x2_0, x1_1, x2_1, ...]
    perm = np.empty(D, np.int64)
    for h in range(H):
        base = h * DK
        perm[base + 0:base + DK:2] = base + np.arange(32)
        perm[base + 1:base + DK:2] = base + 32 + np.arange(32)

    def pack_dr(w):  # [D, C] f32 -> [KP, 128, 2, C] fp8
        C = w.shape[1]
        return np.ascontiguousarray(
            w.reshape(KP, 2, 128, C).transpose(0, 2, 1, 3)).astype(f8)

    common = {
        "cosr": cos_rep, "sinr": sin_sign,
        "wq8": pack_dr(np.asarray(w_q, np.float32)[:, perm]),
        "wk8": pack_dr(np.asarray(w_k, np.float32)[:, perm]),
        "wv8": pack_dr(np.asarray(w_v, np.float32)),
        "wo8": pack_dr(np.asarray(w_o, np.float32) * WSCALE),
        "w1b": np.asarray(w1, np.float32).astype(bf),
        "w2b": np.asarray(w2, np.float32).astype(bf),
        "b1t": np.ascontiguousarray(
            np.asarray(b1, np.float32).reshape(F // 128, 128).T),
        "identb": np.eye(128, dtype=np.float32).astype(bf),
        "onehot": (np.kron(np.eye(H, dtype=np.float32),
                           np.ones((1, 64), np.float32)) * CTXS).astype(bf),
        "bo": np.asarray(b_o, np.float32).reshape(1, D),
        "b2r": np.asarray(b2, np.float32).reshape(1, D),
        "g1": np.asarray(gamma1, np.float32).reshape(1, D),
        "be1": np.asarray(beta1, np.float32).reshape(1, D),
        "g2": np.asarray(gamma2, np.float32).reshape(1, D),
        "be2": np.asarray(beta2, np.float32).reshape(1, D),
    }
    xT8_all = [pack_dr(np.ascontiguousarray(x[b].T)) for b in range(B)]

    in_maps = []
    for c in range(NCORES):
        b, r = c // 4, c % 4
        rows = slice(r * RQ, (r + 1) * RQ)
        m = dict(common)
        m["xT8"] = xT8_all[b]
        m["xq8"] = np.ascontiguousarray(xT8_all[b][:, :, :, rows])
        m["xr"] = np.ascontiguousarray(x[b, rows, :]).astype(bf)
        m["qcos"] = np.ascontiguousarray(cos_rep[:, rows])
        m["qsin"] = np.ascontiguousarray(sin_sign[:, rows])
        in_maps.append(m)

    res = bass_utils.run_bass_kernel_spmd(
        nc, in_maps, core_ids=list(range(NCORES)), trace=_trace)

    out = np.empty((B, L, D), np.float32)
    for c in range(NCORES):
        b, r = c // 4, c % 4
        out[b, r * RQ:(r + 1) * RQ, :] = res.results[c]["y"]
    if _trace:
        kernel.last_exec_time_ns = res.exec_time_ns
    return out
